# revision 17
# baseline (speedup 1.0000x reference)
"""Trainium2 Bass kernel for the masked block-diagonal LSTM net.

Model structure (hardcoded from the problem spec):
  - x_seq [512, 64, 32], recurrent state HID=1088 = 34 blocks x 32.
  - U projections are masked so hidden block j only sees input feature j
    (block 0 additionally sees features 0,1 again via the interaction rows);
    hidden blocks 32,33 receive NO input projection at all.
  - V recurrent matrices are masked block-diagonal -> the 34 blocks evolve
    completely independently through the scan.

Sharding: hidden-block parallel. Cores 0..7 each own 4 input-driven blocks
(128 hidden rows) x the full batch 512. Layout on device is h^T:
[hid on partitions, batch on free dim].

Optimizations (from perfetto/NTFF trace analysis; 454us -> ~307us):
  - The per-step serial cycle h(t) -> gates -> c -> tanh -> h(t+1) paces the
    kernel; engines are not throughput-bound.
  - Input projections (wu) for step t+1 are pre-computed into the next PSUM
    buffer during step t, so only the 4 wv accumulates sit in the cycle and
    the PE stays fed (its DVFS p-state clock decays on every stall).
  - PSUM zero-region semantics: start=True marks the WHOLE 2KB bank
    pending-zero (a later accumulate to pending bytes overwrites them), so
    only the first gate of each bank pair uses start=True; the second
    writes with start=False (pending bytes -> overwrite, clearing the flag).
  - ONE sigmoid per chunk covers all 4 gate banks (ACT per-instruction
    overhead is ~270ns, so fewer/bigger ACT instructions win; the ACT queue
    sits on the serial cycle because tanh(c) must drain it).
  - tanh(c) uses ACT Tanh directly (same HW act table as Sigmoid -> no table
    swap).  g = tanh(y) comes from sigmoid(2y) with weights pre-scaled x2,
    fixed up with one tensor_scalar (2g'-1, ~220ns) instead of the slower
    scalar_tensor_tensor (~770ns).
  - c-update chain runs entirely on the DVE (t1=f*c first); GpSimd tensor
    ops have ~480ns fixed overhead and lose to the DVE even off-queue.
  - t=0 is specialized: h==0/c==0 make the wv accumulates and f*c no-ops.

Blocks 32,33 are bias-only (no x dependence): their state is identical for
every batch element, so their scalar contribution to the readout (and the
tiny 16-feature static MLP + final sigmoid) is folded into the host-side
unshard step.
"""

import sys

sys.path.insert(0, "/opt/trn_rl_repo")

import numpy as np

B = 512
T = 64
INPUT_SZ = 32
HPF = 32
INTER = [(0, 1), (2, 3)]
NB = INPUT_SZ + len(INTER)  # 34
HID = NB * HPF  # 1088
IN_SZ = INPUT_SZ + 2 * len(INTER)  # 36
F_STAT = 16
N_CORES = 8
BLOCKS_PER_CORE = 4
CORE_HID = BLOCKS_PER_CORE * HPF  # 128
CHUNKS = 2  # batch-column chunks per step (pipelining granularity)
CB = B // CHUNKS

_CACHE = {}


def _build_masks():
    um = np.zeros((IN_SZ, HID), np.float32)
    for i in range(INPUT_SZ):
        um[i, i * HPF : (i + 1) * HPF] = 1.0
    for i in range(0, len(INTER), 2):
        um[i + INPUT_SZ, i * HPF : (i + 1) * HPF] = 1.0
        um[i + INPUT_SZ + 1, i * HPF : (i + 1) * HPF] = 1.0
    vm = np.kron(np.eye(NB, dtype=np.float32), np.ones((HPF, HPF), np.float32))
    return um, vm


def _build_program():
    import concourse.bass as bass
    import concourse.tile as tile
    from concourse import bacc, mybir

    f32 = mybir.dt.float32
    f16 = mybir.dt.float16
    u16 = mybir.dt.uint16
    ACT = mybir.ActivationFunctionType

    nc = bacc.Bacc("TRN2", target_bir_lowering=False, debug=False)

    xf_d = nc.dram_tensor("xf", [5, T * B], f16, kind="ExternalInput").ap()
    wu_d = nc.dram_tensor("wu", [4, 5, CORE_HID], f16, kind="ExternalInput").ap()
    wv_d = nc.dram_tensor("wv", [4, CORE_HID, CORE_HID], f16, kind="ExternalInput").ap()
    oc_d = nc.dram_tensor("oc", [CORE_HID, 1], f16, kind="ExternalInput").ap()
    part_d = nc.dram_tensor("partial", [1, B], f32, kind="ExternalOutput").ap()

    with tile.TileContext(nc) as tc:
        with (
            tc.tile_pool(name="const", bufs=1) as cpool,
            tc.tile_pool(name="state", bufs=2) as spool,
            tc.tile_pool(name="work", bufs=3) as wpool,
            tc.tile_pool(name="psum", bufs=2, space="PSUM") as ppool,
        ):
            # column-chunked xf load: each 64KB-per-row transfer takes ~9us,
            # and a single dma_start would gate step 0 on ALL of it.  Chunked,
            # step t only waits for chunk t//8 (subtile deps).
            xf = cpool.tile([5, T * B], f16, tag="xf")
            XNCH = 8
            XCW = T * B // XNCH
            for k in range(XNCH):
                nc.sync.dma_start(
                    xf[:, k * XCW : (k + 1) * XCW],
                    xf_d[:, k * XCW : (k + 1) * XCW],
                )
            wu = []
            wv = []
            for g in range(4):
                wut = cpool.tile([5, CORE_HID], f16, tag=f"wu{g}")
                nc.sync.dma_start(wut[:], wu_d[g])
                wu.append(wut)
                wvt = cpool.tile([CORE_HID, CORE_HID], f16, tag=f"wv{g}")
                nc.sync.dma_start(wvt[:], wv_d[g])
                wv.append(wvt)
            oc = cpool.tile([CORE_HID, 1], f16, tag="oc")
            nc.sync.dma_start(oc[:], oc_d[:])

            hs_t = []
            cs_t = []
            for ch in range(CHUNKS):
                h0 = spool.tile([CORE_HID, CB], f16, tag=f"h{ch}")
                c0 = spool.tile([CORE_HID, CB], f16, tag=f"c{ch}")
                nc.vector.memset(h0[:].bitcast(u16), 0)
                nc.vector.memset(c0[:].bitcast(u16), 0)
                hs_t.append(h0)
                cs_t.append(c0)

            def xcols(t, ch):
                return xf[:, t * B + ch * CB : t * B + (ch + 1) * CB]

            def emit_wu(ps, t, ch):
                for g in range(4):
                    # one start=True per psum BANK (gates pair 2-per-bank):
                    # start marks the whole 2KB zero region pending-zero, so
                    # a second start=True in the same bank would wipe its
                    # neighbor; the second gate writes with start=False
                    # (pending bytes -> overwrite, clearing the flag).
                    nc.tensor.matmul(
                        ps[:, g], wu[g][:], xcols(t, ch),
                        start=(g % 2 == 0), stop=False, skip_group_check=True,
                    )

            ps_cur = []
            for ch in range(CHUNKS):
                ps = ppool.tile([128, 4, CB], f32, tag=f"ps{ch}")
                emit_wu(ps, 0, ch)
                ps_cur.append(ps)

            for t in range(T):
                # --- phase A: recurrent accumulates, then next step's wu ---
                # (t=0: h==0, so the wv accumulates are no-ops -> skip)
                if t > 0:
                    for ch in range(CHUNKS):
                        for g in range(4):
                            nc.tensor.matmul(
                                ps_cur[ch][:, g], wv[g][:], hs_t[ch][:],
                                start=False, stop=True, skip_group_check=True,
                            )
                ps_next = [None] * CHUNKS
                if t + 1 < T:
                    for ch in range(CHUNKS):
                        psn = ppool.tile([128, 4, CB], f32, tag=f"ps{ch}")
                        emit_wu(psn, t + 1, ch)
                        ps_next[ch] = psn

                # --- phase B: one sigmoid over all 4 gate banks/chunk ---
                ifgo_t = []
                for ch in range(CHUNKS):
                    ifgo = wpool.tile([CORE_HID, 4, CB], f16, tag=f"ifgo{ch}")
                    nc.scalar.activation(ifgo[:], ps_cur[ch][:], ACT.Sigmoid)
                    ifgo_t.append(ifgo)

                # --- phase C: c update, all on DVE (t1 first) ---
                # (t=0: c==0 -> c_new = i * g directly)
                cn_t = []
                for ch in range(CHUNKS):
                    i_ = ifgo_t[ch][:, 0]
                    f_ = ifgo_t[ch][:, 1]
                    g_ = ifgo_t[ch][:, 2]  # sigmoid(2y): tanh(y) = 2g'-1
                    if t > 0:
                        t1 = wpool.tile([CORE_HID, CB], f16, tag=f"t1{ch}")
                        nc.vector.tensor_mul(t1[:], f_, cs_t[ch][:])
                    g2 = wpool.tile([CORE_HID, CB], f16, tag=f"g2{ch}")
                    nc.vector.tensor_scalar(
                        g2[:], g_, 2.0, -1.0, mybir.AluOpType.mult,
                        mybir.AluOpType.add,
                    )
                    c_new = spool.tile([CORE_HID, CB], f16, tag=f"c{ch}")
                    if t > 0:
                        t2 = wpool.tile([CORE_HID, CB], f16, tag=f"t2{ch}")
                        nc.vector.tensor_mul(t2[:], i_, g2[:])
                        nc.vector.tensor_add(c_new[:], t1[:], t2[:])
                    else:
                        nc.vector.tensor_mul(c_new[:], i_, g2[:])
                    cn_t.append(c_new)

                # --- phase D: tanh(c) + output gate ---
                for ch in range(CHUNKS):
                    tc_ = wpool.tile([CORE_HID, CB], f16, tag=f"tc{ch}")
                    nc.scalar.activation(tc_[:], cn_t[ch][:], ACT.Tanh)
                    h_new = spool.tile([CORE_HID, CB], f16, tag=f"h{ch}")
                    nc.vector.tensor_mul(h_new[:], ifgo_t[ch][:, 3], tc_[:])
                    hs_t[ch] = h_new
                    cs_t[ch] = cn_t[ch]

                ps_cur = ps_next

            # readout partial: oc^T @ h  -> [1, B]
            outsb = wpool.tile([1, B], f32, tag="outsb")
            for ch in range(CHUNKS):
                pr = ppool.tile([128, 4, CB], f32, tag=f"ps{ch}")
                nc.tensor.matmul(
                    pr[0:1, 0], oc[:], hs_t[ch][:], start=True, stop=True
                )
                nc.vector.tensor_copy(outsb[:, ch * CB : (ch + 1) * CB], pr[0:1, 0])
            nc.sync.dma_start(part_d[:], outsb[:])

    nc.compile()
    return nc


def _pack_inputs(inputs):
    um, vm = _build_masks()
    # host-tail order (i, f, o, c) -- matches _host_tail's gate indexing
    gates = [
        (inputs["U_i"], inputs["V_i"], inputs["b_i"]),
        (inputs["U_f"], inputs["V_f"], inputs["b_f"]),
        (inputs["U_o"], inputs["V_o"], inputs["b_o"]),
        (inputs["U_c"], inputs["V_c"], inputs["b_c"]),
    ]
    Up = [np.asarray(U, np.float32) * um for U, _, _ in gates]
    Vp = [np.asarray(V, np.float32) * vm for _, V, _ in gates]
    bs = [np.asarray(b, np.float32) for _, _, b in gates]
    x_seq = np.asarray(inputs["x_seq"], np.float32)
    out_coef = np.asarray(inputs["out_coef"], np.float32)

    # device gate order: 0:i 1:f 2:g(=c, pre-scaled x2) 3:o
    DEV_ORDER = [0, 1, 3, 2]  # index into the (i,f,o,c) lists above

    in_maps = []
    for core in range(N_CORES):
        feats = list(range(4 * core, 4 * core + 4))
        hs = slice(CORE_HID * core, CORE_HID * (core + 1))
        xf = np.ones((5, T * B), np.float32)
        # column index = t*B + b
        xf[0:4] = x_seq[:, :, feats].transpose(2, 1, 0).reshape(4, T * B)
        wu = np.zeros((4, 5, CORE_HID), np.float32)
        wv = np.zeros((4, CORE_HID, CORE_HID), np.float32)
        for slot, g in enumerate(DEV_ORDER):
            wu[slot, 0:4] = Up[g][feats, hs]
            if core == 0:
                # interaction rows 32,33 multiply x0,x1 -> fold into rows 0,1
                wu[slot, 0] += Up[g][32, hs]
                wu[slot, 1] += Up[g][33, hs]
            wu[slot, 4] = bs[g][hs]
            wv[slot] = Vp[g][hs, hs]
        # cell gate (slot 2) pre-scaled x2: tanh(y) = 2*sigmoid(2y) - 1
        wu[2] *= 2.0
        wv[2] *= 2.0
        in_maps.append(
            {
                "xf": xf.astype(np.float16),
                "wu": wu.astype(np.float16),
                "wv": wv.astype(np.float16),
                "oc": np.ascontiguousarray(out_coef[hs]).astype(np.float16),
            }
        )
    return in_maps, Vp, bs, out_coef


def _host_tail(inputs, partials, Vp, bs, out_coef):
    """Bias-only blocks 32,33 (batch-independent scalar) + static MLP +
    final sigmoid. All exact model math, done during unshard."""
    aux = slice(32 * HPF, HID)  # hid 1024:1088
    h = np.zeros(2 * HPF, np.float32)
    cst = np.zeros(2 * HPF, np.float32)
    Va = [V[aux, aux] for V in Vp]
    ba = [b[aux] for b in bs]

    def sig(x):
        return 1.0 / (1.0 + np.exp(-x))

    for _ in range(T):
        i_t = sig(ba[0] + h @ Va[0])
        f_t = sig(ba[1] + h @ Va[1])
        o_t = sig(ba[2] + h @ Va[2])
        g_t = np.tanh(ba[3] + h @ Va[3])
        cst = f_t * cst + i_t * g_t
        h = o_t * np.tanh(cst)
    s_aux = float(h @ out_coef[aux, 0])

    x_stat = np.asarray(inputs["x_stat"], np.float32)
    W1 = np.asarray(inputs["W1"], np.float32)
    b1 = np.asarray(inputs["b1"], np.float32)
    W2 = np.asarray(inputs["W2"], np.float32)
    b2 = np.asarray(inputs["b2"], np.float32)
    hid = np.maximum(x_stat[:, :, None] * W1[None] + b1[None], 0.0)
    mlp = sig(np.einsum("bfk,fk->bf", hid, W2) + b2)
    mlp_part = mlp @ out_coef[HID:, 0]

    z = partials.sum(axis=0) + s_aux + mlp_part + float(np.asarray(inputs["out_bias"])[0])
    return sig(z).astype(np.float32).reshape(B, 1)


def kernel(**inputs):
    from concourse.bass_utils import run_bass_kernel_spmd

    if "nc" not in _CACHE:
        _CACHE["nc"] = _build_program()
    nc = _CACHE["nc"]

    in_maps, Vp, bs, out_coef = _pack_inputs(inputs)
    res = run_bass_kernel_spmd(nc, in_maps, core_ids=list(range(N_CORES)))
    partials = np.stack([res.results[c]["partial"][0] for c in range(N_CORES)])
    return _host_tail(inputs, partials, Vp, bs, out_coef)


# revision 18
# speedup vs baseline: 1.0038x; 1.0038x over previous
"""Trainium2 Bass kernel for the masked block-diagonal LSTM net.

Model structure (hardcoded from the problem spec):
  - x_seq [512, 64, 32], recurrent state HID=1088 = 34 blocks x 32.
  - U projections are masked so hidden block j only sees input feature j
    (block 0 additionally sees features 0,1 again via the interaction rows);
    hidden blocks 32,33 receive NO input projection at all.
  - V recurrent matrices are masked block-diagonal -> the 34 blocks evolve
    completely independently through the scan.

Sharding: hidden-block parallel. Cores 0..7 each own 4 input-driven blocks
(128 hidden rows) x the full batch 512. Layout on device is h^T:
[hid on partitions, batch on free dim].

Optimizations (from perfetto/NTFF trace analysis; 454us -> ~307us):
  - The per-step serial cycle h(t) -> gates -> c -> tanh -> h(t+1) paces the
    kernel; engines are not throughput-bound.
  - Input projections (wu) for step t+1 are pre-computed into the next PSUM
    buffer during step t, so only the 4 wv accumulates sit in the cycle and
    the PE stays fed (its DVFS p-state clock decays on every stall).
  - PSUM zero-region semantics: start=True marks the WHOLE 2KB bank
    pending-zero (a later accumulate to pending bytes overwrites them), so
    only the first gate of each bank pair uses start=True; the second
    writes with start=False (pending bytes -> overwrite, clearing the flag).
  - ONE sigmoid per chunk covers all 4 gate banks (ACT per-instruction
    overhead is ~270ns, so fewer/bigger ACT instructions win; the ACT queue
    sits on the serial cycle because tanh(c) must drain it).
  - tanh(c) uses ACT Tanh directly (same HW act table as Sigmoid -> no table
    swap).  g = tanh(y) comes from sigmoid(2y) with weights pre-scaled x2,
    fixed up with one tensor_scalar (2g'-1, ~220ns) instead of the slower
    scalar_tensor_tensor (~770ns).
  - c-update chain runs entirely on the DVE (t1=f*c first); GpSimd tensor
    ops have ~480ns fixed overhead and lose to the DVE even off-queue.
  - t=0 is specialized: h==0/c==0 make the wv accumulates and f*c no-ops.

Blocks 32,33 are bias-only (no x dependence): their state is identical for
every batch element, so their scalar contribution to the readout (and the
tiny 16-feature static MLP + final sigmoid) is folded into the host-side
unshard step.
"""

import sys

sys.path.insert(0, "/opt/trn_rl_repo")

import numpy as np

B = 512
T = 64
INPUT_SZ = 32
HPF = 32
INTER = [(0, 1), (2, 3)]
NB = INPUT_SZ + len(INTER)  # 34
HID = NB * HPF  # 1088
IN_SZ = INPUT_SZ + 2 * len(INTER)  # 36
F_STAT = 16
N_CORES = 8
BLOCKS_PER_CORE = 4
CORE_HID = BLOCKS_PER_CORE * HPF  # 128
CHUNKS = 2  # batch-column chunks per step (pipelining granularity)
CB = B // CHUNKS

_CACHE = {}


def _build_masks():
    um = np.zeros((IN_SZ, HID), np.float32)
    for i in range(INPUT_SZ):
        um[i, i * HPF : (i + 1) * HPF] = 1.0
    for i in range(0, len(INTER), 2):
        um[i + INPUT_SZ, i * HPF : (i + 1) * HPF] = 1.0
        um[i + INPUT_SZ + 1, i * HPF : (i + 1) * HPF] = 1.0
    vm = np.kron(np.eye(NB, dtype=np.float32), np.ones((HPF, HPF), np.float32))
    return um, vm


def _build_program():
    import concourse.bass as bass
    import concourse.tile as tile
    from concourse import bacc, mybir

    f32 = mybir.dt.float32
    f16 = mybir.dt.float16
    u16 = mybir.dt.uint16
    ACT = mybir.ActivationFunctionType

    nc = bacc.Bacc("TRN2", target_bir_lowering=False, debug=False)

    xf_d = nc.dram_tensor("xf", [5, T * B], f16, kind="ExternalInput").ap()
    wu_d = nc.dram_tensor("wu", [4, 5, CORE_HID], f16, kind="ExternalInput").ap()
    wv_d = nc.dram_tensor("wv", [4, CORE_HID, CORE_HID], f16, kind="ExternalInput").ap()
    oc_d = nc.dram_tensor("oc", [CORE_HID, 1], f16, kind="ExternalInput").ap()
    part_d = nc.dram_tensor("partial", [1, B], f32, kind="ExternalOutput").ap()

    with tile.TileContext(nc) as tc:
        with (
            tc.tile_pool(name="const", bufs=1) as cpool,
            tc.tile_pool(name="state", bufs=2) as spool,
            tc.tile_pool(name="work", bufs=3) as wpool,
            tc.tile_pool(name="psum", bufs=2, space="PSUM") as ppool,
        ):
            # column-chunked xf load: each 64KB-per-row transfer takes ~9us,
            # and a single dma_start would gate step 0 on ALL of it.  Chunked,
            # step t only waits for chunk t//8 (subtile deps).
            xf = cpool.tile([5, T * B], f16, tag="xf")
            XNCH = 8
            XCW = T * B // XNCH
            for k in range(XNCH):
                nc.sync.dma_start(
                    xf[:, k * XCW : (k + 1) * XCW],
                    xf_d[:, k * XCW : (k + 1) * XCW],
                )
            wu = []
            wv = []
            for g in range(4):
                wut = cpool.tile([5, CORE_HID], f16, tag=f"wu{g}")
                nc.sync.dma_start(wut[:], wu_d[g])
                wu.append(wut)
                wvt = cpool.tile([CORE_HID, CORE_HID], f16, tag=f"wv{g}")
                nc.sync.dma_start(wvt[:], wv_d[g])
                wv.append(wvt)
            oc = cpool.tile([CORE_HID, 1], f16, tag="oc")
            nc.sync.dma_start(oc[:], oc_d[:])

            hs_t = []
            cs_t = []
            for ch in range(CHUNKS):
                h0 = spool.tile([CORE_HID, CB], f16, tag=f"h{ch}")
                c0 = spool.tile([CORE_HID, CB], f16, tag=f"c{ch}")
                nc.vector.memset(h0[:].bitcast(u16), 0)
                nc.vector.memset(c0[:].bitcast(u16), 0)
                hs_t.append(h0)
                cs_t.append(c0)

            def xcols(t, ch):
                return xf[:, t * B + ch * CB : t * B + (ch + 1) * CB]

            # matmul emission order (0,2,1,3) alternates psum BANKS between
            # consecutive matmuls (gates pair 2-per-bank) to dodge same-bank
            # back-to-back write conflicts.
            GORDER = (0, 2, 1, 3)

            def emit_wu(ps, t, ch):
                for g in GORDER:
                    # one start=True per psum BANK: start marks the whole 2KB
                    # zero region pending-zero, so the second gate in a bank
                    # writes with start=False (pending bytes -> overwrite,
                    # clearing the flag).  Order keeps g0 before g1 and g2
                    # before g3 within each bank.
                    nc.tensor.matmul(
                        ps[:, g], wu[g][:], xcols(t, ch),
                        start=(g % 2 == 0), stop=False, skip_group_check=True,
                    )

            ps_cur = []
            for ch in range(CHUNKS):
                ps = ppool.tile([128, 4, CB], f32, tag=f"ps{ch}")
                emit_wu(ps, 0, ch)
                ps_cur.append(ps)

            for t in range(T):
                # --- phase A: recurrent accumulates, then next step's wu ---
                # (t=0: h==0, so the wv accumulates are no-ops -> skip)
                if t > 0:
                    for ch in range(CHUNKS):
                        for g in GORDER:
                            nc.tensor.matmul(
                                ps_cur[ch][:, g], wv[g][:], hs_t[ch][:],
                                start=False, stop=True, skip_group_check=True,
                            )
                ps_next = [None] * CHUNKS
                if t + 1 < T:
                    for ch in range(CHUNKS):
                        psn = ppool.tile([128, 4, CB], f32, tag=f"ps{ch}")
                        emit_wu(psn, t + 1, ch)
                        ps_next[ch] = psn

                # --- phase B: one sigmoid over all 4 gate banks/chunk ---
                ifgo_t = []
                for ch in range(CHUNKS):
                    ifgo = wpool.tile([CORE_HID, 4, CB], f16, tag=f"ifgo{ch}")
                    nc.scalar.activation(ifgo[:], ps_cur[ch][:], ACT.Sigmoid)
                    ifgo_t.append(ifgo)

                # --- phase C: c update, all on DVE (t1 first) ---
                # (t=0: c==0 -> c_new = i * g directly)
                cn_t = []
                for ch in range(CHUNKS):
                    i_ = ifgo_t[ch][:, 0]
                    f_ = ifgo_t[ch][:, 1]
                    g_ = ifgo_t[ch][:, 2]  # sigmoid(2y): tanh(y) = 2g'-1
                    if t > 0:
                        t1 = wpool.tile([CORE_HID, CB], f16, tag=f"t1{ch}")
                        nc.vector.tensor_mul(t1[:], f_, cs_t[ch][:])
                    g2 = wpool.tile([CORE_HID, CB], f16, tag=f"g2{ch}")
                    nc.vector.tensor_scalar(
                        g2[:], g_, 2.0, -1.0, mybir.AluOpType.mult,
                        mybir.AluOpType.add,
                    )
                    c_new = spool.tile([CORE_HID, CB], f16, tag=f"c{ch}")
                    if t > 0:
                        t2 = wpool.tile([CORE_HID, CB], f16, tag=f"t2{ch}")
                        nc.vector.tensor_mul(t2[:], i_, g2[:])
                        nc.vector.tensor_add(c_new[:], t1[:], t2[:])
                    else:
                        nc.vector.tensor_mul(c_new[:], i_, g2[:])
                    cn_t.append(c_new)

                # --- phase D: tanh(c) + output gate ---
                for ch in range(CHUNKS):
                    tc_ = wpool.tile([CORE_HID, CB], f16, tag=f"tc{ch}")
                    nc.scalar.activation(tc_[:], cn_t[ch][:], ACT.Tanh)
                    h_new = spool.tile([CORE_HID, CB], f16, tag=f"h{ch}")
                    nc.vector.tensor_mul(h_new[:], ifgo_t[ch][:, 3], tc_[:])
                    hs_t[ch] = h_new
                    cs_t[ch] = cn_t[ch]

                ps_cur = ps_next

            # readout partial: oc^T @ h  -> [1, B]
            outsb = wpool.tile([1, B], f32, tag="outsb")
            for ch in range(CHUNKS):
                pr = ppool.tile([128, 4, CB], f32, tag=f"ps{ch}")
                nc.tensor.matmul(
                    pr[0:1, 0], oc[:], hs_t[ch][:], start=True, stop=True
                )
                nc.vector.tensor_copy(outsb[:, ch * CB : (ch + 1) * CB], pr[0:1, 0])
            nc.sync.dma_start(part_d[:], outsb[:])

    nc.compile()
    return nc


def _pack_inputs(inputs):
    um, vm = _build_masks()
    # host-tail order (i, f, o, c) -- matches _host_tail's gate indexing
    gates = [
        (inputs["U_i"], inputs["V_i"], inputs["b_i"]),
        (inputs["U_f"], inputs["V_f"], inputs["b_f"]),
        (inputs["U_o"], inputs["V_o"], inputs["b_o"]),
        (inputs["U_c"], inputs["V_c"], inputs["b_c"]),
    ]
    Up = [np.asarray(U, np.float32) * um for U, _, _ in gates]
    Vp = [np.asarray(V, np.float32) * vm for _, V, _ in gates]
    bs = [np.asarray(b, np.float32) for _, _, b in gates]
    x_seq = np.asarray(inputs["x_seq"], np.float32)
    out_coef = np.asarray(inputs["out_coef"], np.float32)

    # device gate order: 0:i 1:f 2:g(=c, pre-scaled x2) 3:o
    DEV_ORDER = [0, 1, 3, 2]  # index into the (i,f,o,c) lists above

    in_maps = []
    for core in range(N_CORES):
        feats = list(range(4 * core, 4 * core + 4))
        hs = slice(CORE_HID * core, CORE_HID * (core + 1))
        xf = np.ones((5, T * B), np.float32)
        # column index = t*B + b
        xf[0:4] = x_seq[:, :, feats].transpose(2, 1, 0).reshape(4, T * B)
        wu = np.zeros((4, 5, CORE_HID), np.float32)
        wv = np.zeros((4, CORE_HID, CORE_HID), np.float32)
        for slot, g in enumerate(DEV_ORDER):
            wu[slot, 0:4] = Up[g][feats, hs]
            if core == 0:
                # interaction rows 32,33 multiply x0,x1 -> fold into rows 0,1
                wu[slot, 0] += Up[g][32, hs]
                wu[slot, 1] += Up[g][33, hs]
            wu[slot, 4] = bs[g][hs]
            wv[slot] = Vp[g][hs, hs]
        # cell gate (slot 2) pre-scaled x2: tanh(y) = 2*sigmoid(2y) - 1
        wu[2] *= 2.0
        wv[2] *= 2.0
        in_maps.append(
            {
                "xf": xf.astype(np.float16),
                "wu": wu.astype(np.float16),
                "wv": wv.astype(np.float16),
                "oc": np.ascontiguousarray(out_coef[hs]).astype(np.float16),
            }
        )
    return in_maps, Vp, bs, out_coef


def _host_tail(inputs, partials, Vp, bs, out_coef):
    """Bias-only blocks 32,33 (batch-independent scalar) + static MLP +
    final sigmoid. All exact model math, done during unshard."""
    aux = slice(32 * HPF, HID)  # hid 1024:1088
    h = np.zeros(2 * HPF, np.float32)
    cst = np.zeros(2 * HPF, np.float32)
    Va = [V[aux, aux] for V in Vp]
    ba = [b[aux] for b in bs]

    def sig(x):
        return 1.0 / (1.0 + np.exp(-x))

    for _ in range(T):
        i_t = sig(ba[0] + h @ Va[0])
        f_t = sig(ba[1] + h @ Va[1])
        o_t = sig(ba[2] + h @ Va[2])
        g_t = np.tanh(ba[3] + h @ Va[3])
        cst = f_t * cst + i_t * g_t
        h = o_t * np.tanh(cst)
    s_aux = float(h @ out_coef[aux, 0])

    x_stat = np.asarray(inputs["x_stat"], np.float32)
    W1 = np.asarray(inputs["W1"], np.float32)
    b1 = np.asarray(inputs["b1"], np.float32)
    W2 = np.asarray(inputs["W2"], np.float32)
    b2 = np.asarray(inputs["b2"], np.float32)
    hid = np.maximum(x_stat[:, :, None] * W1[None] + b1[None], 0.0)
    mlp = sig(np.einsum("bfk,fk->bf", hid, W2) + b2)
    mlp_part = mlp @ out_coef[HID:, 0]

    z = partials.sum(axis=0) + s_aux + mlp_part + float(np.asarray(inputs["out_bias"])[0])
    return sig(z).astype(np.float32).reshape(B, 1)


def kernel(**inputs):
    from concourse.bass_utils import run_bass_kernel_spmd

    if "nc" not in _CACHE:
        _CACHE["nc"] = _build_program()
    nc = _CACHE["nc"]

    in_maps, Vp, bs, out_coef = _pack_inputs(inputs)
    res = run_bass_kernel_spmd(nc, in_maps, core_ids=list(range(N_CORES)))
    partials = np.stack([res.results[c]["partial"][0] for c in range(N_CORES)])
    return _host_tail(inputs, partials, Vp, bs, out_coef)
